# revision 33
# baseline (speedup 1.0000x reference)
"""BLIP3o DiT block on 8 Trainium2 NeuronCores.

Data-parallel over batch (32 -> 4 per core), zero collectives.

v2: mixed-precision PE pipeline tuned from HW microbenchmarks:
  - bf16 operands for all numerically-hot matmuls (same PE streaming rate as
    fp32r, half the DMA/SBUF, faster LDWEIGHTS so N=256 shapes aren't
    load-bound).
  - fp8e4 DoubleRow (K=256/instr, ~2x FLOP rate) for the error-tolerant
    matmuls: eva projection, and all of cross-attention's q2/k2/v2/o2
    (their error washes through softmax normalization). Weight prescales
    (x32 / x64) folded into exp-scale or consumer descales.
  - attention AV+denominator fused in one matmul via [V64|ones64] head
    blocks (out rows 0-63 = AV, 64-127 = replicated denominator).
  - rms2/rms3 square+stats folded into the o1/o2 residual consumers.
  - all weights pre-packed on host into exact SBUF tile layouts so every
    DMA line is contiguous per partition.

Error budget (CPU study): fp8 set {eva,qk2,v2,o2} ~1.26e-2 + bf16 rest
~3e-3 => ~1.3e-2 well under the 2e-2 gate.
"""
import os
import sys
import numpy as np
import ml_dtypes

if "/root/pylocal" not in sys.path:
    sys.path.insert(0, "/root/pylocal")  # antenv.axon_hooks shim (NTFF tracing)
try:
    import antenv
    if "/root/pylocal/antenv" not in list(antenv.__path__):
        antenv.__path__.append("/root/pylocal/antenv")
except Exception:
    pass

import concourse.bass as bass
from concourse import bacc
import concourse.mybir as mybir
from concourse.tile import TileContext
from concourse.bass_utils import run_bass_kernel_spmd

F32 = mybir.dt.float32
F32R = mybir.dt.float32r
BF16 = mybir.dt.bfloat16
F8 = mybir.dt.float8e4
AF = mybir.ActivationFunctionType
OP = mybir.AluOpType
DRMODE = mybir.MatmulPerfMode.DoubleRow
BFNP = ml_dtypes.bfloat16
F8NP = ml_dtypes.float8_e4m3

B, S, L, H, NH, HD, I, E = 32, 256, 256, 1024, 16, 64, 4096, 4096
EPS = 1e-6
GRID = 16
NC_ = 8            # cores
BPC = B // NC_     # batches per core = 4
T = BPC * S        # tokens per core = 1024
HC = H // 128      # 8 feature chunks
EC = E // 128      # 32
IC = I // 128      # 32

WS2 = 32.0         # fp8 prescale for H-fan-in weights (sigma 1/32)
WS4 = 64.0         # fp8 prescale for E-fan-in weights (sigma 1/64)


def _rope_tables():
    q = H // 4
    inv = 1.0 / (10000.0 ** (np.arange(0, q, 2, dtype=np.float64) / q))  # [128]
    pos_x = np.repeat(np.arange(GRID, dtype=np.float64), GRID)  # [S]
    pos_y = np.tile(np.arange(GRID, dtype=np.float64), GRID)
    fx = pos_x[:, None] * inv[None, :128]
    fy = pos_y[:, None] * inv[None, :128]
    t = lambda a: np.ascontiguousarray(
        np.tile(a.T.astype(np.float32), (1, BPC)))  # [128, S] -> [128, T]
    return t(np.cos(fx)), t(np.sin(fx)), t(np.cos(fy)), t(np.sin(fy))


def build_program(debug=False):
    nc = bacc.Bacc()

    # ---------------- DRAM params ----------------
    d = {}
    def P(name, shape, dt, out=False):
        d[name] = nc.declare_dram_parameter(name, list(shape), dt, isOutput=out)
        return d[name]

    hsT_d = P("hsT", [H, T], F32)
    P("encp", [128, 2 * EC * 512], F8)       # [p][half][f][512]
    P("tembT", [H, BPC], F32)
    # stationary-pack layouts [p][o][f][128]
    for w in ["wq1p", "wk1p", "wo1p"]:
        P(w, [128, HC * HC * 128], BF16)
    P("adawp", [128, 48 * HC * 128], BF16)
    P("gatep", [128, IC * HC * 128], BF16)
    P("upp", [128, IC * HC * 128], BF16)
    P("downp", [128, HC * IC * 128], BF16)
    for w in ["wq2p", "wk2p", "wo2p"]:
        P(w, [128, HC * HC * 128], F8)
    P("evawp", [128, HC * EC * 128], F8)
    # moving-pack (vnat) layouts [p][oh][f][512]
    P("wv1p", [128, 2 * HC * 512], BF16)
    P("wv2p", [128, 2 * HC * 512], F8)
    P("eva_bT", [128, HC], F32)
    P("ada_bT", [128, 48], F32)
    P("n1T", [128, HC], F32)
    P("n2T", [128, HC], F32)
    P("n3T", [128, HC], F32)
    for tb in ["cxt", "sxt", "cyt", "syt"]:
        P(tb, [128, T], F32)
    P("ones", [128, 128], F32R)
    P("epsc", [128, 1], F32)
    outT_d = P("outT", [H, T], F32, out=True)
    dbg = {}
    if debug:
        for nm, dt in [("d_modT", F32), ("d_x1T", BF16), ("d_qt", BF16),
                       ("d_kt", BF16), ("d_vp1", BF16), ("d_attnout", BF16),
                       ("d_h1", F32), ("d_evaT", F8), ("d_k2t", BF16),
                       ("d_vp2", BF16), ("d_rms2T", F8), ("d_q2t", BF16),
                       ("d_a2o", F8), ("d_h2", F32), ("d_yT", BF16)]:
            sz = 48 * BPC if nm == "d_modT" else HC * T
            if nm in ("d_vp1", "d_vp2"):
                sz = 2 * BPC * NH * 128
            dbg[nm] = P(nm, [128, sz], dt, out=True)

    def dump(nm, tile):
        if debug:
            nc.sync.dma_start(dbg[nm][:], tile[:])

    # pack views: [p][o][f][m] / [p][oh][f][n]
    stat = lambda name, OC, KC: d[name].rearrange(
        "p (o f m) -> p o f m", o=OC, f=KC)
    mov = lambda name: d[name].rearrange(
        "p (oh f n) -> p oh f n", oh=2, f=HC)

    tc_cm = TileContext(nc)
    tc = tc_cm.__enter__()

    open_pools = {}

    def pool(name, bufs=1, side="left"):
        p = tc.alloc_tile_pool(name=name, bufs=bufs, side=side)
        open_pools[name] = p
        return p

    def free(name):
        open_pools.pop(name).release()

    wpool = pool("wstream", bufs=3)
    sml = pool("sml", bufs=1)
    const = pool("const", bufs=1)
    ps_proj = tc.alloc_tile_pool(name="ps_proj", bufs=3, space="PSUM")
    ps_q = tc.alloc_tile_pool(name="ps_q", bufs=3, space="PSUM")

    def qslot():
        return ps_q.tile([128, 256], F32, tag="q", name="qs")

    # ---------------- constants ----------------
    ones_sb = const.tile([128, 128], F32R)
    nc.sync.dma_start(ones_sb[:], d["ones"][:])
    n_sb = {}
    for w in ["n1T", "n2T", "n3T", "eva_bT", "ada_bT"]:
        n_sb[w] = const.tile([128, d[w].shape[1]], F32, name=w + "_sb")
        nc.sync.dma_start(n_sb[w][:], d[w][:])
    eps_sb = const.tile([128, 1], F32)
    nc.sync.dma_start(eps_sb[:], d["epsc"][:])
    modT = const.tile([128, 48, BPC], F32)
    scale1 = const.tile([128, HC, BPC], F32)
    scale3 = const.tile([128, HC, BPC], F32)

    # hsT loads early (needed by rms1 stats)
    p_hs = pool("p_hs")
    hsT = p_hs.tile([128, HC, T], F32)          # h0 -> h1 -> h2 in place
    hs_r = hsT_d.rearrange("(c p) t -> p c t", p=128)
    for c in range(HC):
        nc.sync.dma_start(hsT[:, c], hs_r[:, c])

    # eva buffers live from the very start: eva's PE work fills the
    # rms1/rope DVE-chain bubble and attn1's exp bubbles.  Their DMAs are
    # issued after mod's first weight tiles so mod can start ASAP.
    p_eva = pool("p_eva")
    evaT = p_eva.tile([128, HC, T], F8)
    p_enc = pool("p_enc")
    ench = p_enc.tile([128, EC, T], F8)
    enc_v = d["encp"].rearrange("p (f t) -> p f t", f=EC)

    p_rope = pool("p_rope")
    rope_t = {}
    for tb in ["cxt", "sxt", "cyt", "syt"]:
        rope_t[tb] = p_rope.tile([128, T], F32, name=tb + "_sb")

    # ---------------- phase 0: modT = (silu(temb) @ ada_w + ada_b)^T ---------
    tembT_sb = const.tile([128, HC, BPC], F32)
    nc.sync.dma_start(tembT_sb[:], d["tembT"].rearrange("(c p) b -> p c b", p=128))
    stemb = const.tile([128, HC, BPC], BF16)
    nc.scalar.activation(stemb[:], tembT_sb[:], AF.Silu)
    ada_v = stat("adawp", 48, HC)

    def mod_chunk(os_):
        with nc.named_scope("mod"):
            for o in os_:
                wt = wpool.tile([128, HC, 128], BF16, tag="w8b", name="ada_t")
                nc.sync.dma_start(wt[:], ada_v[:, o])
                mp = qslot()
                for f in range(HC):
                    nc.tensor.matmul(mp[:, 0:BPC], wt[:, f], stemb[:, f],
                                     start=(f == 0), stop=(f == HC - 1))
                nc.vector.tensor_scalar(modT[:, o, :], mp[:, 0:BPC],
                                        n_sb["ada_bT"][:, o:o + 1], None, OP.add)

    # sh_msa + sc_msa first (gate x1T), rest later
    mod_chunk(range(0, 16))
    for fh in range(2):
        nc.sync.dma_start(ench[:, fh * 16:(fh + 1) * 16], enc_v[:, fh * 16:(fh + 1) * 16])
    for tb in ["cxt", "sxt", "cyt", "syt"]:
        nc.sync.dma_start(rope_t[tb][:], d[tb][:])
    for c in range(HC):
        nc.vector.tensor_scalar(scale1[:, c], modT[:, 8 + c], 1.0,
                                n_sb["n1T"][:, c:c + 1], OP.add, OP.mult)

    # ---------------- rms helpers ----------------
    def stats_pool(name):
        ps_ms = tc.alloc_tile_pool(name="ps_ms_" + name, bufs=1, space="PSUM")
        ms = [ps_ms.tile([128, 512], F32, name=f"ms_{name}_{t}") for t in range(2)]

        def accum(c, t, src_ap):
            sq = sml.tile([128, 512], F32R, tag="sq", bufs=2, name=f"sq_{name}")
            nc.scalar.activation(sq[:], src_ap, AF.Square)
            nc.tensor.matmul(ms[t][:], ones_sb[:], sq[:],
                             start=(c == 0), stop=(c == HC - 1))

        def fin(consumer):
            rt = pool("rt_" + name)
            for t in range(2):
                sroot = rt.tile([128, 512], F32, tag="sroot", bufs=2,
                                name=f"sroot_{name}")
                nc.scalar.activation(sroot[:], ms[t][:], AF.Sqrt,
                                     bias=eps_sb[:, 0:1], scale=1.0 / H)
                invn = rt.tile([128, 512], F32, tag="invn", bufs=2,
                               name=f"invn_{name}")
                nc.vector.reciprocal_approx_fast(invn[:], sroot[:])
                for c in range(HC):
                    xn = rt.tile([128, 512], F32, tag="xn", bufs=4,
                                 name=f"xn_{name}")
                    eng = nc.vector if (c + t) % 2 == 0 else nc.gpsimd
                    eng.tensor_tensor(xn[:], hsT[:, c, t * 512:(t + 1) * 512],
                                      invn[:], OP.mult)
                    consumer(c, t, xn)
            ps_ms.release()
            free("rt_" + name)
        return accum, fin

    # ---------------- phase A: rms1 + modulate + rope -> x1T -----------------
    acc1, fin1 = stats_pool("r1")
    with nc.named_scope("rms1s"):
        for c in range(HC):
            for t in range(2):
                acc1(c, t, hsT[:, c, t * 512:(t + 1) * 512])

    p_x1 = pool("p_x1", side="right")
    x1T = p_x1.tile([128, HC, T], BF16)
    p_xm = pool("p_xm")
    xm = [p_xm.tile([128, T], F32, name=f"xm{i}") for i in range(4)]

    def rms1_consumer(c, t, xn):
        for b2 in range(2):
            b = 2 * t + b2
            if c < 4:
                dst = xm[c][:, b * S:(b + 1) * S]
            else:
                dst = x1T[:, c, b * S:(b + 1) * S]
            nc.vector.tensor_scalar(dst, xn[:, b2 * S:(b2 + 1) * S],
                                    scale1[:, c, b:b + 1],
                                    modT[:, 0 + c, b:b + 1],
                                    OP.mult, OP.add)

    with nc.named_scope("rms1"):
        fin1(rms1_consumer)

    with nc.named_scope("rope"):
        rp = pool("p_ropetmp")
        for (i0, i1, ct, st) in [(0, 1, "cxt", "sxt"), (2, 3, "cyt", "syt")]:
            a, bb = xm[i0], xm[i1]
            t1 = rp.tile([128, T], F32, tag="t1", bufs=2, name="t1")
            t2 = rp.tile([128, T], F32, tag="t2", bufs=2, name="t2")
            nc.vector.tensor_tensor(t1[:], a[:], rope_t[ct][:], OP.mult)
            nc.gpsimd.tensor_tensor(t2[:], bb[:], rope_t[st][:], OP.mult)
            nc.vector.tensor_tensor(x1T[:, i0], t1[:], t2[:], OP.subtract)
            t3 = rp.tile([128, T], F32, tag="t1", bufs=2, name="t3")
            t4 = rp.tile([128, T], F32, tag="t2", bufs=2, name="t4")
            nc.gpsimd.tensor_tensor(t3[:], a[:], rope_t[st][:], OP.mult)
            nc.vector.tensor_tensor(t4[:], bb[:], rope_t[ct][:], OP.mult)
            nc.vector.tensor_tensor(x1T[:, i1], t3[:], t4[:], OP.add)
        free("p_ropetmp")
    free("p_xm")
    free("p_rope")

    # rest of mod overlaps the rope/consumer tail on PE
    mod_chunk(range(16, 48))
    for c in range(HC):
        nc.vector.tensor_scalar(scale3[:, c], modT[:, 32 + c], 1.0,
                                n_sb["n3T"][:, c:c + 1], OP.add, OP.mult)

    # ---------------- helpers ----------------
    def proj(name, w_name, src_sb, consumer, wdt=BF16, OC=HC, KC=HC):
        """Y^T: out[o*128: , tq] accumulated over K.  bf16: N=512 halves;
        fp8 DR: N=256 quarters, f-outer so each stationary pair feeds 4
        consecutive matmuls (LDWEIGHTS amortized 4x)."""
        w_v = stat(w_name, OC, KC)
        dr = (wdt == F8)
        with nc.named_scope(name):
            for o in range(OC):
                wt = wpool.tile([128, KC, 128], wdt,
                                tag="w8" + ("f" if dr else "b"), name=f"{name}_w")
                nc.sync.dma_start(wt[:], w_v[:, o])
                if dr:
                    for half in range(2):
                        ps2 = [qslot() for _ in range(2)]
                        for f in range(KC // 2):
                            for i in range(2):
                                tq = half * 2 + i
                                nc.tensor.matmul(
                                    ps2[i][:], wt[:, 2 * f:2 * f + 2],
                                    src_sb[:, 2 * f:2 * f + 2,
                                           tq * 256:(tq + 1) * 256],
                                    start=(f == 0), stop=(f == KC // 2 - 1),
                                    perf_mode=DRMODE)
                        for i in range(2):
                            consumer(o, half * 2 + i, 256, ps2[i][:])
                else:
                    pt = [ps_proj.tile([128, 512], F32, tag="proj",
                                       name=f"{name}_ps") for _ in range(2)]
                    for f in range(KC):
                        for t in range(2):
                            nc.tensor.matmul(pt[t][:], wt[:, f],
                                             src_sb[:, f, t * 512:(t + 1) * 512],
                                             start=(f == 0), stop=(f == KC - 1))
                    for t in range(2):
                        consumer(o, t, 512, pt[t][:])

    def copy_act(dst, dt_scale=None):
        def c(o, tq, w, p):
            if dt_scale is None:
                nc.scalar.copy(dst[:, o, tq * w:(tq + 1) * w], p)
            else:
                nc.scalar.activation(dst[:, o, tq * w:(tq + 1) * w], p,
                                     AF.Copy, scale=dt_scale)
        return c

    def vnat(w_name, src_sb, dst_v, scope, wdt=BF16, vscale=1.0, side="left"):
        """V natural [tok-chunks, heads] with [V64|ones64] head blocks.
        dst_v: [128, 2*BPC, NH, 128] tile; cols 64:128 pre-set to ones.
        bf16: N=512 tiles (8 head-copies); fp8 DR: N=256 (4 head-copies)."""
        w_v = mov(w_name)
        dr = (wdt == F8)
        wv = pool("wv_" + scope, bufs=2, side=side)

        def head_copies(p, t, j0, nh_):
            for j in range(nh_):
                if j % 2 == 0:
                    nc.scalar.activation(dst_v[:, t, j0 + j, 64:128],
                                         p[:, j * 64:(j + 1) * 64],
                                         AF.Copy, scale=vscale)
                else:
                    nc.vector.tensor_scalar(dst_v[:, t, j0 + j, 64:128],
                                            p[:, j * 64:(j + 1) * 64],
                                            vscale, None, OP.mult)

        with nc.named_scope(scope):
            for oh in range(2):
                wt = wv.tile([128, HC, 512], wdt, tag="wvnat", name=f"{scope}_w")
                nc.sync.dma_start(wt[:], w_v[:, oh])
                if dr:
                    for t in range(2 * BPC):
                        ph = [qslot() for _ in range(2)]
                        for f in range(HC // 2):
                            for hq in range(2):
                                nc.tensor.matmul(
                                    ph[hq][:],
                                    src_sb[:, 2 * f:2 * f + 2,
                                           t * 128:(t + 1) * 128],
                                    wt[:, 2 * f:2 * f + 2,
                                       hq * 256:(hq + 1) * 256],
                                    start=(f == 0), stop=(f == HC // 2 - 1),
                                    perf_mode=DRMODE)
                        for hq in range(2):
                            head_copies(ph[hq][:], t, oh * 8 + hq * 4, 4)
                else:
                    for tp in range(BPC):
                        pt = [ps_proj.tile([128, 512], F32, tag="proj",
                                           name=f"{scope}_ps") for _ in range(2)]
                        for f in range(HC):
                            for i in range(2):
                                t = 2 * tp + i
                                nc.tensor.matmul(
                                    pt[i][:],
                                    src_sb[:, f, t * 128:(t + 1) * 128],
                                    wt[:, f], start=(f == 0),
                                    stop=(f == HC - 1))
                        for i in range(2):
                            head_copies(pt[i][:], 2 * tp + i, oh * 8, 8)
        free("wv_" + scope)

    def attention(qt_sb, kt_sb, vp_sb, out_sb, scope, escale, filler=None):
        attnp = pool("attnp_" + scope, bufs=3, side="right")
        with nc.named_scope(scope):
            for b in range(BPC):
                for hc in range(NH // 2):
                    at = [attnp.tile([128, 2, S], BF16, tag=f"attn{ho}",
                                     name="attn_sb") for ho in range(2)]
                    for kc in range(2):
                        scp = []
                        for ho in range(2):
                            sc_ps = qslot()
                            nc.tensor.matmul(
                                sc_ps[:],
                                kt_sb[ho * 64:(ho + 1) * 64, hc,
                                      b * S + kc * 128: b * S + (kc + 1) * 128],
                                qt_sb[ho * 64:(ho + 1) * 64, hc,
                                      b * S:(b + 1) * S],
                                start=True, stop=True)
                            scp.append(sc_ps)
                        for ho in range(2):
                            nc.scalar.activation(at[ho][:, kc], scp[ho][:],
                                                 AF.Exp, scale=escale)
                    for ho in range(2):
                        h = 2 * hc + ho
                        av = qslot()
                        for kc in range(2):
                            nc.tensor.matmul(av[:],
                                             vp_sb[:, b * 2 + kc, h, :],
                                             at[ho][:, kc],
                                             start=(kc == 0), stop=(kc == 1))
                        inv = attnp.tile([64, S], F32, tag="inv", name="inv_sb")
                        nc.vector.reciprocal_approx_fast(inv[:], av[0:64, :])
                        nc.vector.tensor_tensor(
                            out_sb[ho * 64:(ho + 1) * 64, hc, b * S:(b + 1) * S],
                            av[64:128, :], inv[:], OP.mult)
                    if filler is not None:
                        filler()
        free("attnp_" + scope)

    # eva emission unit: one o-chunk = 16 DR matmuls over full T
    eva_v = stat("evawp", HC, EC)

    def eva_emit(o):
        with nc.named_scope("eva"):
            wt = wpool.tile([128, EC, 128], F8, tag="weva", name="eva_w_t")
            nc.sync.dma_start(wt[:], eva_v[:, o])
            for half in range(2):
                ps2 = [qslot() for _ in range(2)]
                for f in range(EC // 2):
                    for i in range(2):
                        tq = half * 2 + i
                        nc.tensor.matmul(
                            ps2[i][:], wt[:, 2 * f:2 * f + 2],
                            ench[:, 2 * f:2 * f + 2, tq * 256:(tq + 1) * 256],
                            start=(f == 0), stop=(f == EC // 2 - 1),
                            perf_mode=DRMODE)
                for i in range(2):
                    tq = half * 2 + i
                    nc.scalar.activation(evaT[:, o, tq * 256:(tq + 1) * 256],
                                         ps2[i][:], AF.Identity,
                                         bias=n_sb["eva_bT"][:, o:o + 1],
                                         scale=1.0 / WS4)

    # first half of eva fills the PE bubble while DVE runs rms1/rope
    for o in range(4):
        eva_emit(o)

    # ---------------- phase A: V, Q, K, attention, o1 ------------------------
    p_vp = pool("p_vp")
    vp1 = p_vp.tile([128, 2 * BPC, NH, 128], BF16)
    for hh in range(NH):
        nc.gpsimd.memset(vp1[:, :, hh, 0:64], 1.0)
    vnat("wv1p", x1T, vp1, "v1")

    p_qt = pool("p_qt"); qt = p_qt.tile([128, HC, T], BF16)
    p_kt = pool("p_kt"); kt = p_kt.tile([128, HC, T], BF16)
    proj("q1", "wq1p", x1T, copy_act(qt))
    proj("k1", "wk1p", x1T, copy_act(kt))
    free("p_x1")

    dump("d_x1T", x1T); dump("d_qt", qt); dump("d_kt", kt); dump("d_vp1", vp1)
    p_ao = pool("p_ao", side="right")
    attnout = p_ao.tile([128, HC, T], BF16)
    eva_state = {"n": 0, "o": 4}

    def eva_filler():
        eva_state["n"] += 1
        if eva_state["n"] % 8 == 0 and eva_state["o"] < HC:
            eva_emit(eva_state["o"])
            eva_state["o"] += 1

    attention(qt, kt, vp1, attnout, "attn1", float(HD) ** -0.5,
              filler=eva_filler)
    while eva_state["o"] < HC:
        eva_emit(eva_state["o"])
        eva_state["o"] += 1
    free("p_kt"); free("p_qt"); free("p_vp")

    acc2, fin2 = stats_pool("r2")

    def resid_gated_stats(o, t, w, p):
        tg = sml.tile([128, 512], F32, tag="resid", name="resid_t")
        for b2 in range(2):
            b = t * 2 + b2
            nc.vector.tensor_scalar(tg[:, b2 * S:(b2 + 1) * S],
                                    p[:, b2 * S:(b2 + 1) * S],
                                    modT[:, 16 + o, b:b + 1],
                                    None, OP.mult)
        nc.vector.tensor_tensor(hsT[:, o, t * 512:(t + 1) * 512],
                                hsT[:, o, t * 512:(t + 1) * 512],
                                tg[:], OP.add)
        acc2(o, t, hsT[:, o, t * 512:(t + 1) * 512])

    free("p_enc")

    proj("o1", "wo1p", attnout, resid_gated_stats)
    dump("d_h1", hsT)
    free("p_ao")

    # ---------------- phase C: cross attention -------------------------------
    p_ao2 = pool("p_ao2", side="right")
    attn2out = p_ao2.tile([128, HC, T], F8)
    p_k2 = pool("p_k2", side="right"); k2t = p_k2.tile([128, HC, T], BF16)
    proj("k2", "wk2p", evaT, copy_act(k2t), wdt=F8)

    p_v2 = pool("p_v2", side="right")
    vp2 = p_v2.tile([128, 2 * BPC, NH, 128], BF16)
    for hh in range(NH):
        nc.gpsimd.memset(vp2[:, :, hh, 0:64], 1.0)
    vnat("wv2p", evaT, vp2, "v2", wdt=F8, vscale=1.0 / WS2, side="right")
    dump("d_evaT", evaT); dump("d_k2t", k2t); dump("d_vp2", vp2)
    free("p_eva")
    p_r2 = pool("p_r2")
    rms2T = p_r2.tile([128, HC, T], F8)

    def rms2_consumer(c, t, xn):
        nc.vector.tensor_scalar(rms2T[:, c, t * 512:(t + 1) * 512], xn[:],
                                n_sb["n2T"][:, c:c + 1], None, OP.mult)

    with nc.named_scope("rms2"):
        fin2(rms2_consumer)

    p_q2 = pool("p_q2", side="right"); q2t = p_q2.tile([128, HC, T], BF16)
    proj("q2", "wq2p", rms2T, copy_act(q2t), wdt=F8)
    free("p_r2")

    dump("d_rms2T", rms2T); dump("d_q2t", q2t)
    attention(q2t, k2t, vp2, attn2out, "attn2",
              float(HD) ** -0.5 / (WS2 * WS2))
    dump("d_a2o", attn2out)
    free("p_q2"); free("p_v2"); free("p_k2")

    acc3, fin3 = stats_pool("r3")

    def resid_o2_stats(o, tq, w, p):
        tg = sml.tile([128, 256], F32, tag="resid2", name="resid2_t")
        nc.vector.tensor_scalar(tg[:], p[:], 1.0 / WS2, None, OP.mult)
        nc.vector.tensor_tensor(hsT[:, o, tq * 256:(tq + 1) * 256],
                                hsT[:, o, tq * 256:(tq + 1) * 256],
                                tg[:], OP.add)
        if tq % 2 == 1:
            t = tq // 2
            acc3(o, t, hsT[:, o, t * 512:(t + 1) * 512])

    proj("o2", "wo2p", attn2out, resid_o2_stats, wdt=F8)
    dump("d_h2", hsT)
    free("p_ao2")

    # ---------------- phase D: rms3 + MLP ------------------------------------
    wmlp = pool("p_wmlp", bufs=4)
    p_y = pool("p_y")
    yT = p_y.tile([128, HC, T], BF16)

    def rms3_consumer(c, t, xn):
        for b2 in range(2):
            b = 2 * t + b2
            nc.vector.tensor_scalar(yT[:, c, b * S:(b + 1) * S],
                                    xn[:, b2 * S:(b2 + 1) * S],
                                    scale3[:, c, b:b + 1],
                                    modT[:, 24 + c, b:b + 1],
                                    OP.mult, OP.add)

    with nc.named_scope("rms3"):
        fin3(rms3_consumer)

    dump("d_modT", modT); dump("d_yT", yT)
    gate_v = stat("gatep", IC, HC)
    up_v = stat("upp", IC, HC)
    down_v = stat("downp", HC, IC)
    p_mlp = pool("p_mlp", side="right")
    mlpT = p_mlp.tile([128, IC, T], BF16)
    with nc.named_scope("gateup"):
        for o in range(IC):
            wg = wmlp.tile([128, HC, 128], BF16, tag="w8b", name="gate_w_t")
            nc.sync.dma_start(wg[:], gate_v[:, o])
            wu = wmlp.tile([128, HC, 128], BF16, tag="w8b", name="up_w_t")
            nc.sync.dma_start(wu[:], up_v[:, o])
            for t in range(2):
                pg = ps_proj.tile([128, 512], F32, tag="proj", name="g_ps")
                pu = ps_proj.tile([128, 512], F32, tag="proj", name="u_ps")
                for f in range(HC):
                    nc.tensor.matmul(pg[:], wg[:, f],
                                     yT[:, f, t * 512:(t + 1) * 512],
                                     start=(f == 0), stop=(f == HC - 1))
                    nc.tensor.matmul(pu[:], wu[:, f],
                                     yT[:, f, t * 512:(t + 1) * 512],
                                     start=(f == 0), stop=(f == HC - 1))
                gs = sml.tile([128, 512], F32, tag="gsil", name="gsil")
                nc.scalar.activation(gs[:], pg[:], AF.Silu)
                nc.vector.tensor_tensor(mlpT[:, o, t * 512:(t + 1) * 512],
                                        gs[:], pu[:], OP.mult)
    free("p_y")

    out_r = outT_d.rearrange("(c p) t -> p c t", p=128)
    wdn = pool("p_wdown", bufs=3)
    with nc.named_scope("down"):
        for o in range(HC):
            wt = wdn.tile([128, IC, 128], BF16, tag="wdown", name="down_w_t")
            nc.sync.dma_start(wt[:], down_v[:, o])
            pt = [ps_proj.tile([128, 512], F32, tag="proj", name="d_ps")
                  for _ in range(2)]
            for f in range(IC):
                for t in range(2):
                    nc.tensor.matmul(pt[t][:], wt[:, f],
                                     mlpT[:, f, t * 512:(t + 1) * 512],
                                     start=(f == 0), stop=(f == IC - 1))
            for t in range(2):
                ft = sml.tile([128, 512], F32, tag="ft", bufs=3, name="final_t")
                for b2 in range(2):
                    b = t * 2 + b2
                    nc.vector.tensor_scalar(ft[:, b2 * S:(b2 + 1) * S],
                                            pt[t][:, b2 * S:(b2 + 1) * S],
                                            modT[:, 40 + o, b:b + 1],
                                            None, OP.mult)
                nc.vector.tensor_tensor(ft[:], ft[:],
                                        hsT[:, o, t * 512:(t + 1) * 512],
                                        OP.add)
                nc.sync.dma_start(out_r[:, o, t * 512:(t + 1) * 512], ft[:])
    free("p_wdown")
    free("p_wmlp")
    free("p_mlp")

    for nm in reversed(list(open_pools)):
        free(nm)
    ps_q.release(); ps_proj.release()
    tc_cm.__exit__(None, None, None)
    nc.compile()
    return nc


_CACHE = {}


def _get_program(debug=False):
    key = "ncd" if debug else "nc"
    if key not in _CACHE:
        _CACHE[key] = build_program(debug)
    return _CACHE[key]


def _pack_stat(w, KC, OC, dt, scale=1.0):
    """[K, O] -> [128, OC*KC*128] with [p][o][f][m] order."""
    a = np.asarray(w, np.float32) * scale
    a = a.reshape(KC, 128, OC, 128).transpose(1, 2, 0, 3)
    return np.ascontiguousarray(a.reshape(128, -1)).astype(dt)


def _pack_mov(w, KC, dt, scale=1.0):
    """[K, 1024] -> [128, 2*KC*512] with [p][oh][f][n] order."""
    a = np.asarray(w, np.float32) * scale
    a = a.reshape(KC, 128, 2, 512).transpose(1, 2, 0, 3)
    return np.ascontiguousarray(a.reshape(128, -1)).astype(dt)


def kernel(hidden_states, encoder_hidden_states, timestep_emb,
           wq1, wk1, wv1, wo1, wq2, wk2, wv2, wo2,
           eva_w, eva_b, ada_w, ada_b, gate_w, up_w, down_w, n1, n2, n3,
           _trace=False, _debug=False):
    nc = _get_program(_debug)
    f32 = lambda a: np.ascontiguousarray(np.asarray(a), dtype=np.float32)

    cxt, sxt, cyt, syt = _rope_tables()
    colchunks = lambda v, n: np.ascontiguousarray(
        np.asarray(v, np.float32).reshape(n, 128).T)
    shared = dict(
        wq1p=_pack_stat(wq1, HC, HC, BFNP),
        wk1p=_pack_stat(wk1, HC, HC, BFNP),
        wo1p=_pack_stat(wo1, HC, HC, BFNP),
        wq2p=_pack_stat(wq2, HC, HC, F8NP, WS2),
        wk2p=_pack_stat(wk2, HC, HC, F8NP, WS2),
        wo2p=_pack_stat(wo2, HC, HC, F8NP, WS2),
        evawp=_pack_stat(eva_w, EC, HC, F8NP, WS4),
        adawp=_pack_stat(ada_w, HC, 48, BFNP),
        gatep=_pack_stat(gate_w, HC, IC, BFNP),
        upp=_pack_stat(up_w, HC, IC, BFNP),
        downp=_pack_stat(down_w, IC, HC, BFNP),
        wv1p=_pack_mov(wv1, HC, BFNP),
        wv2p=_pack_mov(wv2, HC, F8NP, WS2),
        eva_bT=colchunks(eva_b, HC), ada_bT=colchunks(ada_b, 48),
        n1T=colchunks(n1, HC), n2T=colchunks(n2, HC), n3T=colchunks(n3, HC),
        cxt=cxt, sxt=sxt, cyt=cyt, syt=syt,
        ones=np.ones((128, 128), np.float32),
        epsc=np.full((128, 1), EPS, np.float32),
    )
    hs = f32(hidden_states)
    enc = f32(encoder_hidden_states)
    temb = f32(timestep_emb)

    in_maps = []
    for c in range(NC_):
        sl = slice(c * BPC, (c + 1) * BPC)
        m = dict(shared)
        m["hsT"] = np.ascontiguousarray(hs[sl].transpose(2, 0, 1).reshape(H, T))
        # encp: [p][f][t] from encT [E, T]
        encT = enc[sl].transpose(2, 0, 1).reshape(E, T)
        ep = encT.reshape(EC, 128, T).transpose(1, 0, 2)
        m["encp"] = np.ascontiguousarray(ep.reshape(128, -1)).astype(F8NP)
        m["tembT"] = np.ascontiguousarray(temb[sl].T)
        in_maps.append(m)

    res = run_bass_kernel_spmd(nc, in_maps, core_ids=list(range(NC_)),
                               trace=_trace)
    out = np.empty((B, S, H), np.float32)
    for c in range(NC_):
        o = res.results[c]["outT"]  # [H, T]
        out[c * BPC:(c + 1) * BPC] = np.ascontiguousarray(o.T).reshape(BPC, S, H)
    kernel.last_results = res
    return out


# revision 37
# speedup vs baseline: 1.0013x; 1.0013x over previous
"""BLIP3o DiT block on 8 Trainium2 NeuronCores.

Data-parallel over batch (32 -> 4 per core), zero collectives.

v2: mixed-precision PE pipeline tuned from HW microbenchmarks:
  - bf16 operands for all numerically-hot matmuls (same PE streaming rate as
    fp32r, half the DMA/SBUF, faster LDWEIGHTS so N=256 shapes aren't
    load-bound).
  - fp8e4 DoubleRow (K=256/instr, ~2x FLOP rate) for the error-tolerant
    matmuls: eva projection, and all of cross-attention's q2/k2/v2/o2
    (their error washes through softmax normalization). Weight prescales
    (x32 / x64) folded into exp-scale or consumer descales.
  - attention AV+denominator fused in one matmul via [V64|ones64] head
    blocks (out rows 0-63 = AV, 64-127 = replicated denominator).
  - rms2/rms3 square+stats folded into the o1/o2 residual consumers.
  - all weights pre-packed on host into exact SBUF tile layouts so every
    DMA line is contiguous per partition.

Error budget (CPU study): fp8 set {eva,qk2,v2,o2} ~1.26e-2 + bf16 rest
~3e-3 => ~1.3e-2 well under the 2e-2 gate.
"""
import os
import sys
import numpy as np
import ml_dtypes

if "/root/pylocal" not in sys.path:
    sys.path.insert(0, "/root/pylocal")  # antenv.axon_hooks shim (NTFF tracing)
try:
    import antenv
    if "/root/pylocal/antenv" not in list(antenv.__path__):
        antenv.__path__.append("/root/pylocal/antenv")
except Exception:
    pass

import concourse.bass as bass
from concourse import bacc
import concourse.mybir as mybir
from concourse.tile import TileContext
from concourse.bass_utils import run_bass_kernel_spmd

F32 = mybir.dt.float32
F32R = mybir.dt.float32r
BF16 = mybir.dt.bfloat16
F8 = mybir.dt.float8e4
AF = mybir.ActivationFunctionType
OP = mybir.AluOpType
DRMODE = mybir.MatmulPerfMode.DoubleRow
BFNP = ml_dtypes.bfloat16
F8NP = ml_dtypes.float8_e4m3

B, S, L, H, NH, HD, I, E = 32, 256, 256, 1024, 16, 64, 4096, 4096
EPS = 1e-6
GRID = 16
NC_ = 8            # cores
BPC = B // NC_     # batches per core = 4
T = BPC * S        # tokens per core = 1024
HC = H // 128      # 8 feature chunks
EC = E // 128      # 32
IC = I // 128      # 32

WS2 = 32.0         # fp8 prescale for H-fan-in weights (sigma 1/32)
WS4 = 64.0         # fp8 prescale for E-fan-in weights (sigma 1/64)


def _rope_tables():
    q = H // 4
    inv = 1.0 / (10000.0 ** (np.arange(0, q, 2, dtype=np.float64) / q))  # [128]
    pos_x = np.repeat(np.arange(GRID, dtype=np.float64), GRID)  # [S]
    pos_y = np.tile(np.arange(GRID, dtype=np.float64), GRID)
    fx = pos_x[:, None] * inv[None, :128]
    fy = pos_y[:, None] * inv[None, :128]
    t = lambda a: np.ascontiguousarray(
        np.tile(a.T.astype(np.float32), (1, BPC)))  # [128, S] -> [128, T]
    return t(np.cos(fx)), t(np.sin(fx)), t(np.cos(fy)), t(np.sin(fy))


def build_program(debug=False):
    nc = bacc.Bacc()

    # ---------------- DRAM params ----------------
    d = {}
    def P(name, shape, dt, out=False):
        d[name] = nc.declare_dram_parameter(name, list(shape), dt, isOutput=out)
        return d[name]

    hsT_d = P("hsT", [H, T], F32)
    P("encp", [128, 2 * EC * 512], F8)       # [p][half][f][512]
    P("tembT", [H, BPC], F32)
    # stationary-pack layouts [p][o][f][128]
    for w in ["wq1p", "wk1p", "wo1p"]:
        P(w, [128, HC * HC * 128], BF16)
    P("adawp", [128, 48 * HC * 128], BF16)
    P("gatep", [128, IC * HC * 128], BF16)
    P("upp", [128, IC * HC * 128], BF16)
    P("downp", [128, HC * IC * 128], BF16)
    for w in ["wq2p", "wk2p", "wo2p"]:
        P(w, [128, HC * HC * 128], F8)
    P("evawp", [128, HC * EC * 128], F8)
    # moving-pack (vnat) layouts [p][oh][f][512]
    P("wv1p", [128, 2 * HC * 512], BF16)
    P("wv2p", [128, 2 * HC * 512], F8)
    P("eva_bT", [128, HC], F32)
    P("ada_bT", [128, 48], F32)
    P("n1T", [128, HC], F32)
    P("n2T", [128, HC], F32)
    P("n3T", [128, HC], F32)
    for tb in ["cxt", "sxt", "cyt", "syt"]:
        P(tb, [128, T], F32)
    P("ones", [128, 128], F32R)
    P("epsc", [128, 1], F32)
    outT_d = P("outT", [H, T], F32, out=True)
    dbg = {}
    if debug:
        for nm, dt in [("d_modT", F32), ("d_x1T", BF16), ("d_qt", BF16),
                       ("d_kt", BF16), ("d_vp1", BF16), ("d_attnout", BF16),
                       ("d_h1", F32), ("d_evaT", F8), ("d_k2t", BF16),
                       ("d_vp2", BF16), ("d_rms2T", F8), ("d_q2t", BF16),
                       ("d_a2o", F8), ("d_h2", F32), ("d_yT", BF16)]:
            sz = 48 * BPC if nm == "d_modT" else HC * T
            if nm in ("d_vp1", "d_vp2"):
                sz = 2 * BPC * NH * 128
            dbg[nm] = P(nm, [128, sz], dt, out=True)

    def dump(nm, tile):
        if debug:
            nc.sync.dma_start(dbg[nm][:], tile[:])

    # pack views: [p][o][f][m] / [p][oh][f][n]
    stat = lambda name, OC, KC: d[name].rearrange(
        "p (o f m) -> p o f m", o=OC, f=KC)
    mov = lambda name: d[name].rearrange(
        "p (oh f n) -> p oh f n", oh=2, f=HC)

    tc_cm = TileContext(nc)
    tc = tc_cm.__enter__()

    open_pools = {}

    def pool(name, bufs=1, side="left"):
        p = tc.alloc_tile_pool(name=name, bufs=bufs, side=side)
        open_pools[name] = p
        return p

    def free(name):
        open_pools.pop(name).release()

    wpool = pool("wstream", bufs=3)
    sml = pool("sml", bufs=1)
    const = pool("const", bufs=1)
    ps_proj = tc.alloc_tile_pool(name="ps_proj", bufs=3, space="PSUM")
    ps_q = tc.alloc_tile_pool(name="ps_q", bufs=3, space="PSUM")

    def qslot():
        return ps_q.tile([128, 256], F32, tag="q", name="qs")

    # ---------------- constants ----------------
    ones_sb = const.tile([128, 128], F32R)
    nc.sync.dma_start(ones_sb[:], d["ones"][:])
    n_sb = {}
    for w in ["n1T", "n2T", "n3T", "eva_bT", "ada_bT"]:
        n_sb[w] = const.tile([128, d[w].shape[1]], F32, name=w + "_sb")
        nc.sync.dma_start(n_sb[w][:], d[w][:])
    eps_sb = const.tile([128, 1], F32)
    nc.sync.dma_start(eps_sb[:], d["epsc"][:])
    modT = const.tile([128, 48, BPC], F32)
    scale1 = const.tile([128, HC, BPC], F32)
    scale3 = const.tile([128, HC, BPC], F32)

    # hsT loads early (needed by rms1 stats)
    p_hs = pool("p_hs")
    hsT = p_hs.tile([128, HC, T], F32)          # h0 -> h1 -> h2 in place
    hs_r = hsT_d.rearrange("(c p) t -> p c t", p=128)
    for c in range(HC):
        nc.sync.dma_start(hsT[:, c], hs_r[:, c])

    # eva buffers live from the very start: eva's PE work fills the
    # rms1/rope DVE-chain bubble and attn1's exp bubbles.  Their DMAs are
    # issued after mod's first weight tiles so mod can start ASAP.
    p_eva = pool("p_eva")
    evaT = p_eva.tile([128, HC, T], F8)
    p_enc = pool("p_enc")
    ench = p_enc.tile([128, EC, T], F8)
    enc_v = d["encp"].rearrange("p (f t) -> p f t", f=EC)
    p_wev = pool("p_wev", bufs=1)

    p_rope = pool("p_rope")
    rope_t = {}
    for tb in ["cxt", "sxt", "cyt", "syt"]:
        rope_t[tb] = p_rope.tile([128, T], F32, name=tb + "_sb")

    # ---------------- phase 0: modT = (silu(temb) @ ada_w + ada_b)^T ---------
    tembT_sb = const.tile([128, HC, BPC], F32)
    nc.sync.dma_start(tembT_sb[:], d["tembT"].rearrange("(c p) b -> p c b", p=128))
    stemb = const.tile([128, HC, BPC], BF16)
    nc.scalar.activation(stemb[:], tembT_sb[:], AF.Silu)
    ada_v = stat("adawp", 48, HC)

    def mod_chunk(os_):
        with nc.named_scope("mod"):
            for o in os_:
                wt = wpool.tile([128, HC, 128], BF16, tag="w8b", name="ada_t")
                nc.sync.dma_start(wt[:], ada_v[:, o])
                mp = qslot()
                for f in range(HC):
                    nc.tensor.matmul(mp[:, 0:BPC], wt[:, f], stemb[:, f],
                                     start=(f == 0), stop=(f == HC - 1))
                nc.vector.tensor_scalar(modT[:, o, :], mp[:, 0:BPC],
                                        n_sb["ada_bT"][:, o:o + 1], None, OP.add)

    # sh_msa + sc_msa first (gate x1T), rest later
    mod_chunk(range(0, 16))
    eva_v0 = stat("evawp", HC, EC)
    wev_pref = []
    for fh in range(2):
        nc.sync.dma_start(ench[:, fh * 16:(fh + 1) * 16],
                          enc_v[:, fh * 16:(fh + 1) * 16])
    for o in range(4):
        wt = p_wev.tile([128, EC, 128], F8, name=f"wevp{o}")
        nc.sync.dma_start(wt[:], eva_v0[:, o])
        wev_pref.append(wt)
    for tb in ["cxt", "sxt", "cyt", "syt"]:
        nc.sync.dma_start(rope_t[tb][:], d[tb][:])
    for c in range(HC):
        nc.vector.tensor_scalar(scale1[:, c], modT[:, 8 + c], 1.0,
                                n_sb["n1T"][:, c:c + 1], OP.add, OP.mult)

    # ---------------- rms helpers ----------------
    def stats_pool(name):
        ps_ms = tc.alloc_tile_pool(name="ps_ms_" + name, bufs=1, space="PSUM")
        ms = [ps_ms.tile([128, 512], F32, name=f"ms_{name}_{t}") for t in range(2)]

        def accum(c, t, src_ap):
            sq = sml.tile([128, 512], F32R, tag="sq", bufs=2, name=f"sq_{name}")
            nc.scalar.activation(sq[:], src_ap, AF.Square)
            nc.tensor.matmul(ms[t][:], ones_sb[:], sq[:],
                             start=(c == 0), stop=(c == HC - 1))

        def fin(consumer):
            rt = pool("rt_" + name)
            for t in range(2):
                sroot = rt.tile([128, 512], F32, tag="sroot", bufs=2,
                                name=f"sroot_{name}")
                nc.scalar.activation(sroot[:], ms[t][:], AF.Sqrt,
                                     bias=eps_sb[:, 0:1], scale=1.0 / H)
                invn = rt.tile([128, 512], F32, tag="invn", bufs=2,
                               name=f"invn_{name}")
                nc.vector.reciprocal_approx_fast(invn[:], sroot[:])
                for c in range(HC):
                    xn = rt.tile([128, 512], F32, tag="xn", bufs=4,
                                 name=f"xn_{name}")
                    eng = nc.vector if (c + t) % 2 == 0 else nc.gpsimd
                    eng.tensor_tensor(xn[:], hsT[:, c, t * 512:(t + 1) * 512],
                                      invn[:], OP.mult)
                    consumer(c, t, xn)
            ps_ms.release()
            free("rt_" + name)
        return accum, fin

    # ---------------- phase A: rms1 + modulate + rope -> x1T -----------------
    acc1, fin1 = stats_pool("r1")
    with nc.named_scope("rms1s"):
        for c in range(HC):
            for t in range(2):
                acc1(c, t, hsT[:, c, t * 512:(t + 1) * 512])

    p_x1 = pool("p_x1", side="right")
    x1T = p_x1.tile([128, HC, T], BF16)
    p_xm = pool("p_xm")
    xm = [p_xm.tile([128, T], F32, name=f"xm{i}") for i in range(4)]

    def rms1_consumer(c, t, xn):
        for b2 in range(2):
            b = 2 * t + b2
            if c < 4:
                dst = xm[c][:, b * S:(b + 1) * S]
            else:
                dst = x1T[:, c, b * S:(b + 1) * S]
            nc.vector.tensor_scalar(dst, xn[:, b2 * S:(b2 + 1) * S],
                                    scale1[:, c, b:b + 1],
                                    modT[:, 0 + c, b:b + 1],
                                    OP.mult, OP.add)

    with nc.named_scope("rms1"):
        fin1(rms1_consumer)

    with nc.named_scope("rope"):
        rp = pool("p_ropetmp")
        for (i0, i1, ct, st) in [(0, 1, "cxt", "sxt"), (2, 3, "cyt", "syt")]:
            a, bb = xm[i0], xm[i1]
            t1 = rp.tile([128, T], F32, tag="t1", bufs=2, name="t1")
            t2 = rp.tile([128, T], F32, tag="t2", bufs=2, name="t2")
            nc.vector.tensor_tensor(t1[:], a[:], rope_t[ct][:], OP.mult)
            nc.gpsimd.tensor_tensor(t2[:], bb[:], rope_t[st][:], OP.mult)
            nc.vector.tensor_tensor(x1T[:, i0], t1[:], t2[:], OP.subtract)
            t3 = rp.tile([128, T], F32, tag="t1", bufs=2, name="t3")
            t4 = rp.tile([128, T], F32, tag="t2", bufs=2, name="t4")
            nc.gpsimd.tensor_tensor(t3[:], a[:], rope_t[st][:], OP.mult)
            nc.vector.tensor_tensor(t4[:], bb[:], rope_t[ct][:], OP.mult)
            nc.vector.tensor_tensor(x1T[:, i1], t3[:], t4[:], OP.add)
        free("p_ropetmp")
    free("p_xm")
    free("p_rope")



    # ---------------- helpers ----------------
    def proj(name, w_name, src_sb, consumer, wdt=BF16, OC=HC, KC=HC):
        """Y^T: out[o*128: , tq] accumulated over K.  bf16: N=512 halves;
        fp8 DR: N=256 quarters, f-outer so each stationary pair feeds 4
        consecutive matmuls (LDWEIGHTS amortized 4x)."""
        w_v = stat(w_name, OC, KC)
        dr = (wdt == F8)
        with nc.named_scope(name):
            for o in range(OC):
                wt = wpool.tile([128, KC, 128], wdt,
                                tag="w8" + ("f" if dr else "b"), name=f"{name}_w")
                nc.sync.dma_start(wt[:], w_v[:, o])
                if dr:
                    for half in range(2):
                        ps2 = [qslot() for _ in range(2)]
                        for f in range(KC // 2):
                            for i in range(2):
                                tq = half * 2 + i
                                nc.tensor.matmul(
                                    ps2[i][:], wt[:, 2 * f:2 * f + 2],
                                    src_sb[:, 2 * f:2 * f + 2,
                                           tq * 256:(tq + 1) * 256],
                                    start=(f == 0), stop=(f == KC // 2 - 1),
                                    perf_mode=DRMODE)
                        for i in range(2):
                            consumer(o, half * 2 + i, 256, ps2[i][:])
                else:
                    pt = [ps_proj.tile([128, 512], F32, tag="proj",
                                       name=f"{name}_ps") for _ in range(2)]
                    for f in range(KC):
                        for t in range(2):
                            nc.tensor.matmul(pt[t][:], wt[:, f],
                                             src_sb[:, f, t * 512:(t + 1) * 512],
                                             start=(f == 0), stop=(f == KC - 1))
                    for t in range(2):
                        consumer(o, t, 512, pt[t][:])

    def copy_act(dst, dt_scale=None):
        def c(o, tq, w, p):
            if dt_scale is None:
                nc.scalar.copy(dst[:, o, tq * w:(tq + 1) * w], p)
            else:
                nc.scalar.activation(dst[:, o, tq * w:(tq + 1) * w], p,
                                     AF.Copy, scale=dt_scale)
        return c

    def vnat(w_name, src_sb, dst_v, scope, wdt=BF16, vscale=1.0, side="left"):
        """V natural [tok-chunks, heads] with [V64|ones64] head blocks.
        dst_v: [128, 2*BPC, NH, 128] tile; cols 64:128 pre-set to ones.
        bf16: N=512 tiles (8 head-copies); fp8 DR: N=256 (4 head-copies)."""
        w_v = mov(w_name)
        dr = (wdt == F8)
        wv = pool("wv_" + scope, bufs=2, side=side)

        def head_copies(p, t, j0, nh_):
            for j in range(nh_):
                if j % 2 == 0:
                    nc.scalar.activation(dst_v[:, t, j0 + j, 64:128],
                                         p[:, j * 64:(j + 1) * 64],
                                         AF.Copy, scale=vscale)
                else:
                    nc.vector.tensor_scalar(dst_v[:, t, j0 + j, 64:128],
                                            p[:, j * 64:(j + 1) * 64],
                                            vscale, None, OP.mult)

        with nc.named_scope(scope):
            for oh in range(2):
                wt = wv.tile([128, HC, 512], wdt, tag="wvnat", name=f"{scope}_w")
                nc.sync.dma_start(wt[:], w_v[:, oh])
                if dr:
                    for t in range(2 * BPC):
                        ph = [qslot() for _ in range(2)]
                        for f in range(HC // 2):
                            for hq in range(2):
                                nc.tensor.matmul(
                                    ph[hq][:],
                                    src_sb[:, 2 * f:2 * f + 2,
                                           t * 128:(t + 1) * 128],
                                    wt[:, 2 * f:2 * f + 2,
                                       hq * 256:(hq + 1) * 256],
                                    start=(f == 0), stop=(f == HC // 2 - 1),
                                    perf_mode=DRMODE)
                        for hq in range(2):
                            head_copies(ph[hq][:], t, oh * 8 + hq * 4, 4)
                else:
                    for tp in range(BPC):
                        pt = [ps_proj.tile([128, 512], F32, tag="proj",
                                           name=f"{scope}_ps") for _ in range(2)]
                        for f in range(HC):
                            for i in range(2):
                                t = 2 * tp + i
                                nc.tensor.matmul(
                                    pt[i][:],
                                    src_sb[:, f, t * 128:(t + 1) * 128],
                                    wt[:, f], start=(f == 0),
                                    stop=(f == HC - 1))
                        for i in range(2):
                            head_copies(pt[i][:], 2 * tp + i, oh * 8, 8)
        free("wv_" + scope)

    def attention(qt_sb, kt_sb, vp_sb, out_sb, scope, escale, filler=None):
        attnp = pool("attnp_" + scope, bufs=3, side="right")
        with nc.named_scope(scope):
            for b in range(BPC):
                for hc in range(NH // 2):
                    at = [attnp.tile([128, 2, S], BF16, tag=f"attn{ho}",
                                     name="attn_sb") for ho in range(2)]
                    for kc in range(2):
                        scp = []
                        for ho in range(2):
                            sc_ps = qslot()
                            nc.tensor.matmul(
                                sc_ps[:],
                                kt_sb[ho * 64:(ho + 1) * 64, hc,
                                      b * S + kc * 128: b * S + (kc + 1) * 128],
                                qt_sb[ho * 64:(ho + 1) * 64, hc,
                                      b * S:(b + 1) * S],
                                start=True, stop=True)
                            scp.append(sc_ps)
                        for ho in range(2):
                            nc.scalar.activation(at[ho][:, kc], scp[ho][:],
                                                 AF.Exp, scale=escale)
                    for ho in range(2):
                        h = 2 * hc + ho
                        av = qslot()
                        for kc in range(2):
                            nc.tensor.matmul(av[:],
                                             vp_sb[:, b * 2 + kc, h, :],
                                             at[ho][:, kc],
                                             start=(kc == 0), stop=(kc == 1))
                        inv = attnp.tile([64, S], F32, tag="inv", name="inv_sb")
                        nc.vector.reciprocal_approx_fast(inv[:], av[0:64, :])
                        nc.vector.tensor_tensor(
                            out_sb[ho * 64:(ho + 1) * 64, hc, b * S:(b + 1) * S],
                            av[64:128, :], inv[:], OP.mult)
                    if filler is not None:
                        filler()
        free("attnp_" + scope)

    # eva emission unit: one o-chunk = 16 DR matmuls over full T
    eva_v = stat("evawp", HC, EC)

    def eva_emit(o):
        with nc.named_scope("eva"):
            if o < 4:
                wt = wev_pref[o]
            else:
                wt = wpool.tile([128, EC, 128], F8, tag="weva", name="eva_w_t")
                nc.sync.dma_start(wt[:], eva_v[:, o])
            for half in range(2):
                ps2 = [qslot() for _ in range(2)]
                for f in range(EC // 2):
                    for i in range(2):
                        tq = half * 2 + i
                        nc.tensor.matmul(
                            ps2[i][:], wt[:, 2 * f:2 * f + 2],
                            ench[:, 2 * f:2 * f + 2, tq * 256:(tq + 1) * 256],
                            start=(f == 0), stop=(f == EC // 2 - 1),
                            perf_mode=DRMODE)
                for i in range(2):
                    tq = half * 2 + i
                    nc.scalar.activation(evaT[:, o, tq * 256:(tq + 1) * 256],
                                         ps2[i][:], AF.Identity,
                                         bias=n_sb["eva_bT"][:, o:o + 1],
                                         scale=1.0 / WS4)

    # first half of eva fills the PE bubble while DVE runs rms1/rope
    for o in range(4):
        eva_emit(o)
    free("p_wev")
    mod_chunk(range(16, 48))
    for c in range(HC):
        nc.vector.tensor_scalar(scale3[:, c], modT[:, 32 + c], 1.0,
                                n_sb["n3T"][:, c:c + 1], OP.add, OP.mult)

    # ---------------- phase A: V, Q, K, attention, o1 ------------------------
    p_vp = pool("p_vp")
    vp1 = p_vp.tile([128, 2 * BPC, NH, 128], BF16)
    for hh in range(NH):
        nc.gpsimd.memset(vp1[:, :, hh, 0:64], 1.0)
    vnat("wv1p", x1T, vp1, "v1")

    p_qt = pool("p_qt"); qt = p_qt.tile([128, HC, T], BF16)
    p_kt = pool("p_kt"); kt = p_kt.tile([128, HC, T], BF16)
    proj("q1", "wq1p", x1T, copy_act(qt))
    proj("k1", "wk1p", x1T, copy_act(kt))
    free("p_x1")

    dump("d_x1T", x1T); dump("d_qt", qt); dump("d_kt", kt); dump("d_vp1", vp1)
    p_ao = pool("p_ao", side="right")
    attnout = p_ao.tile([128, HC, T], BF16)
    eva_state = {"n": 0, "o": 4}

    def eva_filler():
        eva_state["n"] += 1
        if eva_state["n"] % 8 == 0 and eva_state["o"] < HC:
            eva_emit(eva_state["o"])
            eva_state["o"] += 1

    attention(qt, kt, vp1, attnout, "attn1", float(HD) ** -0.5,
              filler=eva_filler)
    while eva_state["o"] < HC:
        eva_emit(eva_state["o"])
        eva_state["o"] += 1
    free("p_kt"); free("p_qt"); free("p_vp")

    acc2, fin2 = stats_pool("r2")

    def resid_gated_stats(o, t, w, p):
        tg = sml.tile([128, 512], F32, tag="resid", name="resid_t")
        for b2 in range(2):
            b = t * 2 + b2
            nc.vector.tensor_scalar(tg[:, b2 * S:(b2 + 1) * S],
                                    p[:, b2 * S:(b2 + 1) * S],
                                    modT[:, 16 + o, b:b + 1],
                                    None, OP.mult)
        nc.vector.tensor_tensor(hsT[:, o, t * 512:(t + 1) * 512],
                                hsT[:, o, t * 512:(t + 1) * 512],
                                tg[:], OP.add)
        acc2(o, t, hsT[:, o, t * 512:(t + 1) * 512])

    free("p_enc")

    proj("o1", "wo1p", attnout, resid_gated_stats)
    dump("d_h1", hsT)
    free("p_ao")

    # ---------------- phase C: cross attention -------------------------------
    p_ao2 = pool("p_ao2", side="right")
    attn2out = p_ao2.tile([128, HC, T], F8)
    p_k2 = pool("p_k2", side="right"); k2t = p_k2.tile([128, HC, T], BF16)
    proj("k2", "wk2p", evaT, copy_act(k2t), wdt=F8)

    p_v2 = pool("p_v2", side="right")
    vp2 = p_v2.tile([128, 2 * BPC, NH, 128], BF16)
    for hh in range(NH):
        nc.gpsimd.memset(vp2[:, :, hh, 0:64], 1.0)

    # rms2's DVE chain runs while v2 streams on the PE
    p_r2 = pool("p_r2")
    rms2T = p_r2.tile([128, HC, T], F8)

    def rms2_consumer(c, t, xn):
        nc.vector.tensor_scalar(rms2T[:, c, t * 512:(t + 1) * 512], xn[:],
                                n_sb["n2T"][:, c:c + 1], None, OP.mult)

    with nc.named_scope("rms2"):
        fin2(rms2_consumer)

    vnat("wv2p", evaT, vp2, "v2", wdt=F8, vscale=1.0 / WS2, side="right")
    dump("d_evaT", evaT); dump("d_k2t", k2t); dump("d_vp2", vp2)

    p_q2 = pool("p_q2", side="right"); q2t = p_q2.tile([128, HC, T], BF16)
    proj("q2", "wq2p", rms2T, copy_act(q2t), wdt=F8)
    dump("d_rms2T", rms2T); dump("d_q2t", q2t)
    free("p_r2")
    free("p_eva")

    acc3, fin3 = stats_pool("r3")
    wo2_v = stat("wo2p", HC, HC)

    def o2_quarter(tq):
        with nc.named_scope("o2"):
            for o in range(HC):
                wt = wpool.tile([128, HC, 128], F8, tag="w8f", name="o2_w")
                nc.sync.dma_start(wt[:], wo2_v[:, o])
                p = qslot()
                for f in range(HC // 2):
                    nc.tensor.matmul(
                        p[:], wt[:, 2 * f:2 * f + 2],
                        attn2out[:, 2 * f:2 * f + 2, tq * 256:(tq + 1) * 256],
                        start=(f == 0), stop=(f == HC // 2 - 1),
                        perf_mode=DRMODE)
                tg = sml.tile([128, 256], F32, tag="resid2", name="resid2_t")
                nc.vector.tensor_scalar(tg[:], p[:], 1.0 / WS2, None, OP.mult)
                nc.vector.tensor_tensor(hsT[:, o, tq * 256:(tq + 1) * 256],
                                        hsT[:, o, tq * 256:(tq + 1) * 256],
                                        tg[:], OP.add)
                if tq % 2 == 1:
                    t = tq // 2
                    acc3(o, t, hsT[:, o, t * 512:(t + 1) * 512])

    o2_state = {"n": 0, "tq": 0}

    def o2_filler():
        o2_state["n"] += 1
        if o2_state["n"] % 8 == 0 and o2_state["tq"] < 4:
            o2_quarter(o2_state["tq"])
            o2_state["tq"] += 1

    attention(q2t, k2t, vp2, attn2out, "attn2",
              float(HD) ** -0.5 / (WS2 * WS2), filler=o2_filler)
    dump("d_a2o", attn2out)
    while o2_state["tq"] < 4:
        o2_quarter(o2_state["tq"])
        o2_state["tq"] += 1
    free("p_q2"); free("p_v2"); free("p_k2")
    dump("d_h2", hsT)
    free("p_ao2")

    # ---------------- phase D: rms3 + MLP ------------------------------------
    wmlp = pool("p_wmlp", bufs=4)
    p_y = pool("p_y")
    yT = p_y.tile([128, HC, T], BF16)

    def rms3_consumer(c, t, xn):
        for b2 in range(2):
            b = 2 * t + b2
            nc.vector.tensor_scalar(yT[:, c, b * S:(b + 1) * S],
                                    xn[:, b2 * S:(b2 + 1) * S],
                                    scale3[:, c, b:b + 1],
                                    modT[:, 24 + c, b:b + 1],
                                    OP.mult, OP.add)

    with nc.named_scope("rms3"):
        fin3(rms3_consumer)

    dump("d_modT", modT); dump("d_yT", yT)
    gate_v = stat("gatep", IC, HC)
    up_v = stat("upp", IC, HC)
    down_v = stat("downp", HC, IC)
    p_mlp = pool("p_mlp", side="right")
    mlpT = p_mlp.tile([128, IC, T], BF16)
    with nc.named_scope("gateup"):
        for o in range(IC):
            wg = wmlp.tile([128, HC, 128], BF16, tag="w8b", name="gate_w_t")
            nc.sync.dma_start(wg[:], gate_v[:, o])
            wu = wmlp.tile([128, HC, 128], BF16, tag="w8b", name="up_w_t")
            nc.sync.dma_start(wu[:], up_v[:, o])
            for t in range(2):
                pg = ps_proj.tile([128, 512], F32, tag="proj", name="g_ps")
                pu = ps_proj.tile([128, 512], F32, tag="proj", name="u_ps")
                for f in range(HC):
                    nc.tensor.matmul(pg[:], wg[:, f],
                                     yT[:, f, t * 512:(t + 1) * 512],
                                     start=(f == 0), stop=(f == HC - 1))
                    nc.tensor.matmul(pu[:], wu[:, f],
                                     yT[:, f, t * 512:(t + 1) * 512],
                                     start=(f == 0), stop=(f == HC - 1))
                gs = sml.tile([128, 512], F32, tag="gsil", name="gsil")
                nc.scalar.activation(gs[:], pg[:], AF.Silu)
                nc.vector.tensor_tensor(mlpT[:, o, t * 512:(t + 1) * 512],
                                        gs[:], pu[:], OP.mult)
    free("p_y")

    out_r = outT_d.rearrange("(c p) t -> p c t", p=128)
    wdn = pool("p_wdown", bufs=3)
    with nc.named_scope("down"):
        for o in range(HC):
            wt = wdn.tile([128, IC, 128], BF16, tag="wdown", name="down_w_t")
            nc.sync.dma_start(wt[:], down_v[:, o])
            pt = [ps_proj.tile([128, 512], F32, tag="proj", name="d_ps")
                  for _ in range(2)]
            for f in range(IC):
                for t in range(2):
                    nc.tensor.matmul(pt[t][:], wt[:, f],
                                     mlpT[:, f, t * 512:(t + 1) * 512],
                                     start=(f == 0), stop=(f == IC - 1))
            for t in range(2):
                ft = sml.tile([128, 512], F32, tag="ft", bufs=3, name="final_t")
                for b2 in range(2):
                    b = t * 2 + b2
                    nc.vector.tensor_scalar(ft[:, b2 * S:(b2 + 1) * S],
                                            pt[t][:, b2 * S:(b2 + 1) * S],
                                            modT[:, 40 + o, b:b + 1],
                                            None, OP.mult)
                nc.vector.tensor_tensor(ft[:], ft[:],
                                        hsT[:, o, t * 512:(t + 1) * 512],
                                        OP.add)
                nc.sync.dma_start(out_r[:, o, t * 512:(t + 1) * 512], ft[:])
    free("p_wdown")
    free("p_wmlp")
    free("p_mlp")

    for nm in reversed(list(open_pools)):
        free(nm)
    ps_q.release(); ps_proj.release()
    tc_cm.__exit__(None, None, None)
    nc.compile()
    return nc


_CACHE = {}


def _get_program(debug=False):
    key = "ncd" if debug else "nc"
    if key not in _CACHE:
        _CACHE[key] = build_program(debug)
    return _CACHE[key]


def _pack_stat(w, KC, OC, dt, scale=1.0):
    """[K, O] -> [128, OC*KC*128] with [p][o][f][m] order."""
    a = np.asarray(w, np.float32) * scale
    a = a.reshape(KC, 128, OC, 128).transpose(1, 2, 0, 3)
    return np.ascontiguousarray(a.reshape(128, -1)).astype(dt)


def _pack_mov(w, KC, dt, scale=1.0):
    """[K, 1024] -> [128, 2*KC*512] with [p][oh][f][n] order."""
    a = np.asarray(w, np.float32) * scale
    a = a.reshape(KC, 128, 2, 512).transpose(1, 2, 0, 3)
    return np.ascontiguousarray(a.reshape(128, -1)).astype(dt)


def kernel(hidden_states, encoder_hidden_states, timestep_emb,
           wq1, wk1, wv1, wo1, wq2, wk2, wv2, wo2,
           eva_w, eva_b, ada_w, ada_b, gate_w, up_w, down_w, n1, n2, n3,
           _trace=False, _debug=False):
    nc = _get_program(_debug)
    f32 = lambda a: np.ascontiguousarray(np.asarray(a), dtype=np.float32)

    cxt, sxt, cyt, syt = _rope_tables()
    colchunks = lambda v, n: np.ascontiguousarray(
        np.asarray(v, np.float32).reshape(n, 128).T)
    shared = dict(
        wq1p=_pack_stat(wq1, HC, HC, BFNP),
        wk1p=_pack_stat(wk1, HC, HC, BFNP),
        wo1p=_pack_stat(wo1, HC, HC, BFNP),
        wq2p=_pack_stat(wq2, HC, HC, F8NP, WS2),
        wk2p=_pack_stat(wk2, HC, HC, F8NP, WS2),
        wo2p=_pack_stat(wo2, HC, HC, F8NP, WS2),
        evawp=_pack_stat(eva_w, EC, HC, F8NP, WS4),
        adawp=_pack_stat(ada_w, HC, 48, BFNP),
        gatep=_pack_stat(gate_w, HC, IC, BFNP),
        upp=_pack_stat(up_w, HC, IC, BFNP),
        downp=_pack_stat(down_w, IC, HC, BFNP),
        wv1p=_pack_mov(wv1, HC, BFNP),
        wv2p=_pack_mov(wv2, HC, F8NP, WS2),
        eva_bT=colchunks(eva_b, HC), ada_bT=colchunks(ada_b, 48),
        n1T=colchunks(n1, HC), n2T=colchunks(n2, HC), n3T=colchunks(n3, HC),
        cxt=cxt, sxt=sxt, cyt=cyt, syt=syt,
        ones=np.ones((128, 128), np.float32),
        epsc=np.full((128, 1), EPS, np.float32),
    )
    hs = f32(hidden_states)
    enc = f32(encoder_hidden_states)
    temb = f32(timestep_emb)

    in_maps = []
    for c in range(NC_):
        sl = slice(c * BPC, (c + 1) * BPC)
        m = dict(shared)
        m["hsT"] = np.ascontiguousarray(hs[sl].transpose(2, 0, 1).reshape(H, T))
        # encp: [p][f][t] from encT [E, T]
        encT = enc[sl].transpose(2, 0, 1).reshape(E, T)
        ep = encT.reshape(EC, 128, T).transpose(1, 0, 2)
        m["encp"] = np.ascontiguousarray(ep.reshape(128, -1)).astype(F8NP)
        m["tembT"] = np.ascontiguousarray(temb[sl].T)
        in_maps.append(m)

    res = run_bass_kernel_spmd(nc, in_maps, core_ids=list(range(NC_)),
                               trace=_trace)
    out = np.empty((B, S, H), np.float32)
    for c in range(NC_):
        o = res.results[c]["outT"]  # [H, T]
        out[c * BPC:(c + 1) * BPC] = np.ascontiguousarray(o.T).reshape(BPC, S, H)
    kernel.last_results = res
    return out
